# revision 1
# baseline (speedup 1.0000x reference)
"""Trainium2 Bass kernel for nn_GatedFeedForward (gated feed-forward with
feature attention).

Reference computation per batch b (B=8, N=4096, D=1024):
    VR = x @ Wvr.T + bvr ; VI = x @ Wvi.T + bvi
    V  = VR * tanh(softplus(VI))
    K  = x @ Wk.T + bk   ; Q  = x @ Wq.T + bq
    Kn = K / (||K||_col + 1e-5) ; Qn = Q / (||Q||_col + 1e-5)   (norm over N)
    A  = smu(Kn.T @ Qn)          # smu ~ leaky-relu(slope 0.25) for mu=1e6
    out = V @ A

Sharding: pure data-parallel over batch — 8 batches on 8 NeuronCores, one
batch per core, no collectives.

Math simplifications used by the kernel:
  * smu(x) = 0.5*((1+a)x + (1-a)x*erf(1e6*(1-a)x)) == 0.625x + 0.375|x|
    to within fp32 rounding for |x| >~ 5e-6 (erf saturates); the difference
    for tiny |x| is O(1e-6) absolute and vanishes in the D-sum.
  * leaky is positively homogeneous, so with rk=1/(||K||+1e-5), rq likewise:
        A = leaky((K.T Q) * rk[d] * rq[e]) = rk[d]*rq[e]*leaky(K.T Q)
    rk folds into A's rows (per-partition scale), rq folds into the final
    output tiles (free-dim broadcast multiply).

Kernel plan per core (all matmuls bf16 with fp32 PSUM accumulation):
  Pass 1 (per 512-row chunk of the sequence): K,Q = x@W, accumulate
      G += K_c.T Q_c in SBUF fp32, and column norms nk2/nq2 via an
      ones-vector matmul over squared K/Q tiles.
  Mid: rk/rq from norms; A' = rk[d] * (0.625 G + 0.375|G|) in bf16.
  Pass 2 (per chunk): VR,VI = x@W, V^T = VR * tanh(softplus(VI)),
      out_chunk = (V^T).T @ A' scaled by rq[e], DMA to DRAM.

Inputs are host-prepared: x[b] transposed to [D, N] bf16 (so the d
contraction dim lands on SBUF partitions), weights transposed to [D, D]
([in,out]) bf16. Biases are structurally zero for this problem (spec fill:
zeros); a host-side numpy fallback handles the never-expected nonzero case.
"""

import numpy as np
import ml_dtypes

import concourse.bass as bass
import concourse.tile as tile
from concourse import bacc, mybir
from concourse.bass import ts

F32 = mybir.dt.float32
BF16 = mybir.dt.bfloat16

B = 8
N_FULL = 4096
D_FULL = 1024
N_CORES = 8

P = 128  # SBUF partitions
NC = 512  # sequence chunk
EF = 512  # free-dim span per matmul / psum bank


def build_program(n=N_FULL, d=D_FULL):
    """Build the single-core SPMD Bass program for one [n, d] batch."""
    assert n % NC == 0 and d % P == 0
    n_chunks = n // NC
    n_sub = NC // P  # 128-row subtiles per chunk
    n_dblk = d // P  # contraction blocks
    ef = min(EF, d)
    n_ef = d // ef  # free-dim spans of the feature dim

    nc = bacc.Bacc("TRN2", target_bir_lowering=False, debug=False,
                   num_devices=N_CORES)
    xt = nc.dram_tensor("xt", [d, n], BF16, kind="ExternalInput")
    wvr = nc.dram_tensor("wvr", [d, d], BF16, kind="ExternalInput")
    wvi = nc.dram_tensor("wvi", [d, d], BF16, kind="ExternalInput")
    wk = nc.dram_tensor("wk", [d, d], BF16, kind="ExternalInput")
    wq = nc.dram_tensor("wq", [d, d], BF16, kind="ExternalInput")
    out_d = nc.dram_tensor("out", [n, d], F32, kind="ExternalOutput")

    with tile.TileContext(nc) as tc:
        with tc.tile_pool(name="const", bufs=1) as const_pool, \
             tc.tile_pool(name="weights", bufs=1) as w_pool, \
             tc.tile_pool(name="post", bufs=1) as post_pool, \
             tc.tile_pool(name="apost", bufs=1) as ap_pool:
            ones = const_pool.tile([P, 1], BF16, name="ones", tag="ones")
            nc.vector.memset(ones, 1.0)

            w_tiles = {}
            for wname, wdram in (("wk", wk), ("wq", wq), ("wvr", wvr), ("wvi", wvi)):
                tl = []
                for db in range(n_dblk):
                    t = w_pool.tile([P, d], BF16, name=f"{wname}{db}", tag=f"{wname}{db}")
                    nc.sync.dma_start(out=t, in_=wdram[ts(db, P), :])
                    tl.append(t)
                w_tiles[wname] = tl

            # ---------------- Pass 1: K, Q -> G, norms ----------------
            with tc.tile_pool(name="xt1", bufs=2) as xt_pool, \
                 tc.tile_pool(name="kq_sb", bufs=1) as kq_sb_pool, \
                 tc.tile_pool(name="sq_sb", bufs=1) as sq_pool, \
                 tc.tile_pool(name="gacc", bufs=1) as g_pool, \
                 tc.tile_pool(name="nrm", bufs=1) as nrm_pool:

                g_tiles = [
                    g_pool.tile([P, d], F32, name=f"g{db}", tag=f"g{db}")
                    for db in range(n_dblk)
                ]
                nk2 = nrm_pool.tile([1, d], F32, name="nk2", tag="nk2")
                nq2 = nrm_pool.tile([1, d], F32, name="nq2", tag="nq2")

                with tc.tile_pool(name="kq_ps", bufs=5, space="PSUM") as kq_ps, \
                     tc.tile_pool(name="g_ps", bufs=2, space="PSUM") as g_ps, \
                     tc.tile_pool(name="nrm_ps", bufs=1, space="PSUM") as nrm_ps:
                    for c in range(n_chunks):
                        xts = []
                        for db in range(n_dblk):
                            t = xt_pool.tile([P, NC], BF16, name=f"x1_{db}", tag=f"x1_{db}")
                            nc.sync.dma_start(out=t, in_=xt[ts(db, P), ts(c, NC)])
                            xts.append(t)

                        k_sb, q_sb, k_sq, q_sq = [], [], [], []
                        for s in range(n_sub):
                            kt = kq_sb_pool.tile([P, d], BF16, name=f"k{s}", tag=f"k{s}")
                            qt = kq_sb_pool.tile([P, d], BF16, name=f"q{s}", tag=f"q{s}")
                            kst = sq_pool.tile([P, d], BF16, name=f"ksq{s}", tag=f"ksq{s}")
                            qst = sq_pool.tile([P, d], BF16, name=f"qsq{s}", tag=f"qsq{s}")
                            for e in range(n_ef):
                                pk = kq_ps.tile([P, ef], F32, name="pk", tag="kqps")
                                pq = kq_ps.tile([P, ef], F32, name="pq", tag="kqps")
                                for db in range(n_dblk):
                                    st = dict(start=(db == 0), stop=(db == n_dblk - 1))
                                    lhsT = xts[db][:, ts(s, P)]
                                    nc.tensor.matmul(pk, lhsT=lhsT, rhs=w_tiles["wk"][db][:, ts(e, ef)], **st)
                                    nc.tensor.matmul(pq, lhsT=lhsT, rhs=w_tiles["wq"][db][:, ts(e, ef)], **st)
                                nc.vector.tensor_copy(out=kt[:, ts(e, ef)], in_=pk)
                                nc.vector.tensor_copy(out=qt[:, ts(e, ef)], in_=pq)
                                nc.scalar.activation(out=kst[:, ts(e, ef)], in_=pk,
                                                     func=mybir.ActivationFunctionType.Square)
                                nc.scalar.activation(out=qst[:, ts(e, ef)], in_=pq,
                                                     func=mybir.ActivationFunctionType.Square)
                            k_sb.append(kt)
                            q_sb.append(qt)
                            k_sq.append(kst)
                            q_sq.append(qst)

                        # G[d, e] += sum_n K[n, d] * Q[n, e]
                        for db in range(n_dblk):
                            for e in range(n_ef):
                                gp = g_ps.tile([P, ef], F32, name="gp", tag="gps")
                                for s in range(n_sub):
                                    nc.tensor.matmul(gp, lhsT=k_sb[s][:, ts(db, P)],
                                                     rhs=q_sb[s][:, ts(e, ef)],
                                                     start=(s == 0), stop=(s == n_sub - 1))
                                if c == 0:
                                    nc.vector.tensor_copy(out=g_tiles[db][:, ts(e, ef)], in_=gp)
                                else:
                                    nc.vector.tensor_add(out=g_tiles[db][:, ts(e, ef)],
                                                         in0=g_tiles[db][:, ts(e, ef)], in1=gp)

                        # column norms: nk2[e] += sum_n K[n, e]^2
                        for sq_list, acc in ((k_sq, nk2), (q_sq, nq2)):
                            for e in range(n_ef):
                                pn = nrm_ps.tile([1, ef], F32, name="pn", tag="nrmps")
                                for s in range(n_sub):
                                    nc.tensor.matmul(pn, lhsT=ones, rhs=sq_list[s][:, ts(e, ef)],
                                                     start=(s == 0), stop=(s == n_sub - 1))
                                if c == 0:
                                    nc.vector.tensor_copy(out=acc[:, ts(e, ef)], in_=pn)
                                else:
                                    nc.vector.tensor_add(out=acc[:, ts(e, ef)],
                                                         in0=acc[:, ts(e, ef)], in1=pn)

                # ---------------- Mid: rk, rq, A' ----------------
                nk = post_pool.tile([1, d], F32, name="nk", tag="nk")
                nq = post_pool.tile([1, d], F32, name="nq", tag="nq")
                nc.scalar.activation(out=nk, in_=nk2, func=mybir.ActivationFunctionType.Sqrt)
                nc.scalar.activation(out=nq, in_=nq2, func=mybir.ActivationFunctionType.Sqrt)
                nc.vector.tensor_scalar_add(out=nk, in0=nk, scalar1=1e-5)
                nc.vector.tensor_scalar_add(out=nq, in0=nq, scalar1=1e-5)
                nc.vector.reciprocal(out=nk, in_=nk)
                nc.vector.reciprocal(out=nq, in_=nq)

                # rk as per-partition columns [P, n_dblk] (via 1-deep matmuls
                # that load each 128-wide rk slice as stationary weights), and
                # rq broadcast across partitions [P, d] (rank-1 outer product
                # with a ones row). Internal-DRAM round trips don't load on
                # this runtime, so both transposes stay on the PE.
                one11 = post_pool.tile([1, 1], F32, name="one11", tag="one11")
                nc.vector.memset(one11, 1.0)
                ones_row = post_pool.tile([1, P], F32, name="ones_row", tag="ones_row")
                nc.vector.memset(ones_row, 1.0)

                rk_col = post_pool.tile([P, n_dblk], F32, name="rk_col", tag="rk_col")
                rq_bc = post_pool.tile([P, d], F32, name="rq_bc", tag="rq_bc")
                with tc.tile_pool(name="misc_ps", bufs=2, space="PSUM") as misc_ps:
                    for db in range(n_dblk):
                        pt = misc_ps.tile([P, 1], F32, name="pt", tag="miscps")
                        nc.tensor.matmul(pt, lhsT=nk[0:1, ts(db, P)], rhs=one11,
                                         start=True, stop=True)
                        nc.vector.tensor_copy(out=rk_col[:, db:db + 1], in_=pt)
                    for e in range(n_ef):
                        pb = misc_ps.tile([P, ef], F32, name="pb", tag="miscps_b")
                        nc.tensor.matmul(pb, lhsT=ones_row, rhs=nq[0:1, ts(e, ef)],
                                         start=True, stop=True)
                        nc.vector.tensor_copy(out=rq_bc[:, ts(e, ef)], in_=pb)

                rk625 = post_pool.tile([P, n_dblk], F32, name="rk625", tag="rk625")
                rk375 = post_pool.tile([P, n_dblk], F32, name="rk375", tag="rk375")
                nc.vector.tensor_scalar_mul(out=rk625, in0=rk_col, scalar1=0.625)
                nc.vector.tensor_scalar_mul(out=rk375, in0=rk_col, scalar1=0.375)

                # A'[d, e] = rk[d] * (0.625 G + 0.375 |G|), bf16
                a_tiles = []
                with tc.tile_pool(name="tabs", bufs=2) as tabs_pool:
                    for db in range(n_dblk):
                        at = ap_pool.tile([P, d], BF16, name=f"a{db}", tag=f"a{db}")
                        tabs = tabs_pool.tile([P, d], F32, name="tabs", tag="tabs")
                        nc.scalar.activation(out=tabs, in_=g_tiles[db],
                                             func=mybir.ActivationFunctionType.Abs,
                                             scale=rk375[:, db:db + 1])
                        nc.vector.scalar_tensor_tensor(out=at, in0=g_tiles[db],
                                                       scalar=rk625[:, db:db + 1], in1=tabs,
                                                       op0=mybir.AluOpType.mult,
                                                       op1=mybir.AluOpType.add)
                        a_tiles.append(at)

            # ---------------- Pass 2: V, output ----------------
            with tc.tile_pool(name="xt2", bufs=2) as xt2_pool, \
                 tc.tile_pool(name="vt", bufs=2) as vt_pool, \
                 tc.tile_pool(name="gate", bufs=2) as gate_pool, \
                 tc.tile_pool(name="osb", bufs=3) as osb_pool, \
                 tc.tile_pool(name="vrvi_ps", bufs=4, space="PSUM") as vrvi_ps, \
                 tc.tile_pool(name="out_ps", bufs=3, space="PSUM") as out_ps:
                for c in range(n_chunks):
                    xts = []
                    for db in range(n_dblk):
                        t = xt2_pool.tile([P, NC], BF16, name=f"x2_{db}", tag=f"x2_{db}")
                        nc.sync.dma_start(out=t, in_=xt[ts(db, P), ts(c, NC)])
                        xts.append(t)

                    # V^T[e, n] tiles, e on partitions
                    vts = []
                    for eb in range(n_dblk):
                        pvr = vrvi_ps.tile([P, NC], F32, name="pvr", tag="vrvi")
                        pvi = vrvi_ps.tile([P, NC], F32, name="pvi", tag="vrvi")
                        for db in range(n_dblk):
                            st = dict(start=(db == 0), stop=(db == n_dblk - 1))
                            nc.tensor.matmul(pvr, lhsT=w_tiles["wvr"][db][:, ts(eb, P)], rhs=xts[db], **st)
                            nc.tensor.matmul(pvi, lhsT=w_tiles["wvi"][db][:, ts(eb, P)], rhs=xts[db], **st)
                        # gate = tanh(softplus(vi)); with s = sigmoid(vi),
                        # m = (1-s)^2:  gate = (1-m)/(1+m) = 2/(1+m) - 1.
                        # (softplus is not in any TRN2 activation table.)
                        sg = gate_pool.tile([P, NC], F32, name="sg", tag="sg")
                        nc.scalar.activation(out=sg, in_=pvi, func=mybir.ActivationFunctionType.Sigmoid)
                        m = gate_pool.tile([P, NC], F32, name="m", tag="m")
                        nc.scalar.activation(out=m, in_=sg, func=mybir.ActivationFunctionType.Square,
                                             scale=-1.0, bias=1.0)
                        nc.vector.tensor_scalar_add(out=m, in0=m, scalar1=1.0)
                        nc.vector.reciprocal(out=m, in_=m)  # r = 1/(1+m)
                        # V = VR * (2r - 1)
                        v2 = gate_pool.tile([P, NC], F32, name="v2", tag="v2")
                        nc.vector.scalar_tensor_tensor(out=v2, in0=pvr, scalar=2.0, in1=m,
                                                       op0=mybir.AluOpType.mult,
                                                       op1=mybir.AluOpType.mult)
                        vt = vt_pool.tile([P, NC], BF16, name=f"vt{eb}", tag=f"vt{eb}")
                        nc.vector.tensor_tensor(out=vt, in0=v2, in1=pvr,
                                                op=mybir.AluOpType.subtract)
                        vts.append(vt)

                    # out[n, e] = rq[e] * sum_d V[n, d] A'[d, e]
                    for s in range(n_sub):
                        for e in range(n_ef):
                            po = out_ps.tile([P, ef], F32, name="po", tag="ops")
                            for db in range(n_dblk):
                                nc.tensor.matmul(po, lhsT=vts[db][:, ts(s, P)],
                                                 rhs=a_tiles[db][:, ts(e, ef)],
                                                 start=(db == 0), stop=(db == n_dblk - 1))
                            ot = osb_pool.tile([P, ef], F32, name="ot", tag="osb")
                            nc.vector.tensor_mul(out=ot, in0=po, in1=rq_bc[:, ts(e, ef)])
                            nc.sync.dma_start(
                                out=out_d[c * NC + s * P:c * NC + (s + 1) * P, ts(e, ef)],
                                in_=ot)
    nc.compile()
    return nc


_PROGRAM_CACHE = {}


def _get_program(n, d):
    key = (n, d)
    if key not in _PROGRAM_CACHE:
        _PROGRAM_CACHE[key] = build_program(n, d)
    return _PROGRAM_CACHE[key]


def _numpy_reference(x, Wvr, bvr, Wvi, bvi, Wk, bk, Wq, bq):
    """Slow fp32 fallback (never expected to run: biases are zeros)."""
    out = np.empty_like(x)
    for b in range(x.shape[0]):
        xb = x[b].astype(np.float64)
        vr = xb @ Wvr.T.astype(np.float64) + bvr
        vi = xb @ Wvi.T.astype(np.float64) + bvi
        v = vr * np.tanh(np.logaddexp(0.0, vi))
        k = xb @ Wk.T.astype(np.float64) + bk
        q = xb @ Wq.T.astype(np.float64) + bq
        kn = k / (np.linalg.norm(k, axis=0, keepdims=True) + 1e-5)
        qn = q / (np.linalg.norm(q, axis=0, keepdims=True) + 1e-5)
        g = kn.T @ qn
        a = 0.625 * g + 0.375 * np.abs(g)
        out[b] = (v @ a).astype(np.float32)
    return out


def kernel(_run_kwargs=None, **inputs):
    run_kwargs = _run_kwargs or {}
    x = np.asarray(inputs["x"], dtype=np.float32)
    Wvr = np.asarray(inputs["Wvr"], dtype=np.float32)
    Wvi = np.asarray(inputs["Wvi"], dtype=np.float32)
    Wk = np.asarray(inputs["Wk"], dtype=np.float32)
    Wq = np.asarray(inputs["Wq"], dtype=np.float32)
    bvr, bvi = np.asarray(inputs["bvr"]), np.asarray(inputs["bvi"])
    bk, bq = np.asarray(inputs["bk"]), np.asarray(inputs["bq"])

    if any(np.any(b != 0) for b in (bvr, bvi, bk, bq)):
        return _numpy_reference(x, Wvr, bvr, Wvi, bvi, Wk, bk, Wq, bq)

    b, n, d = x.shape
    assert b == B and n == N_FULL and d == D_FULL, (b, n, d)

    bf16 = ml_dtypes.bfloat16
    wvr_t = np.ascontiguousarray(Wvr.T).astype(bf16)
    wvi_t = np.ascontiguousarray(Wvi.T).astype(bf16)
    wk_t = np.ascontiguousarray(Wk.T).astype(bf16)
    wq_t = np.ascontiguousarray(Wq.T).astype(bf16)

    in_maps = []
    for i in range(N_CORES):
        in_maps.append({
            "xt": np.ascontiguousarray(x[i].T).astype(bf16),
            "wvr": wvr_t, "wvi": wvi_t, "wk": wk_t, "wq": wq_t,
        })

    nc = _get_program(n, d)
    from concourse.bass_utils import run_bass_kernel_spmd
    res = run_bass_kernel_spmd(nc, in_maps, core_ids=list(range(N_CORES)), **run_kwargs)
    out = np.stack([res.results[i]["out"] for i in range(N_CORES)], axis=0)
    if run_kwargs:
        kernel.last_results = res
    return out



# revision 5
# speedup vs baseline: 1.5319x; 1.5319x over previous
"""Trainium2 Bass kernel for nn_GatedFeedForward (gated feed-forward with
feature attention).

Reference computation per batch b (B=8, N=4096, D=1024):
    VR = x @ Wvr.T ; VI = x @ Wvi.T
    V  = VR * tanh(softplus(VI))
    K  = x @ Wk.T  ; Q  = x @ Wq.T
    Kn = K / (||K||_col + 1e-5) ; Qn = Q / (||Q||_col + 1e-5)   (norm over N)
    A  = smu(Kn.T @ Qn)          # smu == leaky-relu(slope 0.25) at mu=1e6
    out = V @ A

Sharding: pure data-parallel over batch — 8 batches on 8 NeuronCores.

Math restructure vs the naive 6-GEMM formulation (saves ~25% PE work):
  * Gram trick: K^T Q = Wk (x^T x) Wq^T.  With S = x^T x computed once
    (one N-contraction GEMM), K and Q are never materialized:
        T' = S Wk^T ; R' = S Wq^T          (two 1024^3 GEMMs)
        G  = T'^T Wq^T                     (one 1024^3 GEMM; S symmetric)
    replaces K = xWk^T, Q = xWq^T (two N D^2 GEMMs) + K^T Q (one more).
  * Column norms from diagonals: ||K_d||^2 = diag(Wk S Wk^T)[d]
        = sum_i T'[i,d] * Wk^T[i,d]  -> elementwise product + ones-matmul.
  * smu == leaky_relu(0.25) exactly (erf saturates); leaky is positively
    homogeneous so rk folds into A rows (Prelu scale) and rq into the final
    output columns.
  * gate = tanh(softplus(vi)) computed as Tanh(Ln(Exp(vi) + 1)) — three
    chained scalar-engine activations (batched per chunk so only two
    activation-table loads per chunk), keeping the slow DVE reciprocal
    out of the inner loop entirely.

All matmuls bf16 with fp32 PSUM accumulation.
"""

import numpy as np
import ml_dtypes

import concourse.bass as bass
import concourse.tile as tile
from concourse import bacc, mybir
from concourse.bass import ts

F32 = mybir.dt.float32
BF16 = mybir.dt.bfloat16
AF = mybir.ActivationFunctionType

B = 8
N_FULL = 4096
D_FULL = 1024
N_CORES = 8

P = 128   # SBUF partitions
NC = 512  # pass-2 sequence chunk
EF = 512  # free-dim span per matmul / psum bank


def build_program(n=N_FULL, d=D_FULL):
    """Build the single-core SPMD Bass program for one [n, d] batch."""
    assert n % NC == 0 and d % P == 0
    n_chunks = n // NC          # 8
    n_dblk = d // P             # 8 feature blocks
    n_ef = d // EF              # 2 spans of the feature dim
    n_xt = n // P               # 32 row-tiles of x

    nc = bacc.Bacc("TRN2", target_bir_lowering=False, debug=False,
                   num_devices=N_CORES)
    xn_d = nc.dram_tensor("xn", [n, d], BF16, kind="ExternalInput")
    xt_d = nc.dram_tensor("xt", [d, n], BF16, kind="ExternalInput")
    wkq_d = nc.dram_tensor("wkq", [d, 2 * d], BF16, kind="ExternalInput")
    wvr_d = nc.dram_tensor("wvr", [d, d], BF16, kind="ExternalInput")
    wvi_d = nc.dram_tensor("wvi", [d, d], BF16, kind="ExternalInput")
    out_d = nc.dram_tensor("out", [n, d], F32, kind="ExternalOutput")

    with tile.TileContext(nc) as tc:
        with tc.tile_pool(name="const", bufs=1) as const_pool, \
             tc.tile_pool(name="wv", bufs=1) as wv_pool, \
             tc.tile_pool(name="sbf", bufs=1) as sbf_pool, \
             tc.tile_pool(name="abf", bufs=1) as abf_pool, \
             tc.tile_pool(name="post", bufs=1) as post_pool:

            ones_col = const_pool.tile([P, 1], BF16, name="ones_col", tag="ones_col")
            nc.vector.memset(ones_col, 1.0)
            one11 = const_pool.tile([1, 1], F32, name="one11", tag="one11")
            nc.vector.memset(one11, 1.0)
            ones_row = const_pool.tile([1, P], F32, name="ones_row", tag="ones_row")
            nc.vector.memset(ones_row, 1.0)

            s_bf = [sbf_pool.tile([P, d], BF16, name=f"s{j}", tag=f"s{j}")
                    for j in range(n_dblk)]
            a_bf = [abf_pool.tile([P, d], BF16, name=f"a{j}", tag=f"a{j}")
                    for j in range(n_dblk)]

            # ---------------- Pass 1: S = x^T x ----------------
            with tc.tile_pool(name="wkq", bufs=1) as wkq_pool:
                with tc.tile_pool(name="xtile", bufs=1) as xn_pool:
                    # x DMAs first — S matmuls need them immediately; weights
                    # follow in the queues (not used until P1.5/P2).
                    xn = []
                    for ch in range(n_xt):
                        t = xn_pool.tile([P, d], BF16, name=f"x{ch}", tag=f"x{ch}")
                        nc.sync.dma_start(out=t, in_=xn_d[ts(ch, P), :])
                        xn.append(t)
                    wkq = []
                    for dc in range(n_dblk):
                        t = wkq_pool.tile([P, 2 * d], BF16, name=f"wkq{dc}", tag=f"wkq{dc}")
                        nc.sync.dma_start(out=t, in_=wkq_d[ts(dc, P), :])
                        wkq.append(t)
                    wvr, wvi = [], []
                    for wname, wdram, wl in (("wvr", wvr_d, wvr), ("wvi", wvi_d, wvi)):
                        for dc in range(n_dblk):
                            t = wv_pool.tile([P, d], BF16, name=f"{wname}{dc}", tag=f"{wname}{dc}")
                            nc.sync.dma_start(out=t, in_=wdram[ts(dc, P), :])
                            wl.append(t)

                    with tc.tile_pool(name="s_ps", bufs=8, space="PSUM") as s_ps:
                        for wave in range(2):
                            ibs = [4 * wave + k for k in range(4)]
                            ps = {}
                            for ib in ibs:
                                for es in range(n_ef):
                                    ps[(ib, es)] = s_ps.tile([P, EF], F32, name="sps",
                                                             tag="sps")
                            for ch in range(n_xt):
                                for ib in ibs:
                                    lh = xn[ch][:, ts(ib, P)]
                                    for es in range(n_ef):
                                        nc.tensor.matmul(ps[(ib, es)], lhsT=lh,
                                                         rhs=xn[ch][:, ts(es, EF)],
                                                         start=(ch == 0), stop=(ch == n_xt - 1))
                            for ib in ibs:
                                for es in range(n_ef):
                                    nc.vector.tensor_copy(out=s_bf[ib][:, ts(es, EF)],
                                                          in_=ps[(ib, es)])

                # ---------------- Pass 1.5: T'/R', norms, G -> A ----------
                nk2 = post_pool.tile([1, d], F32, name="nk2", tag="nk2")
                nq2 = post_pool.tile([1, d], F32, name="nq2", tag="nq2")
                rk_col = post_pool.tile([P, n_dblk], F32, name="rk_col", tag="rk_col")
                rq_bc = post_pool.tile([P, d], F32, name="rq_bc", tag="rq_bc")

                with tc.tile_pool(name="tp", bufs=1) as tp_pool, \
                     tc.tile_pool(name="prod", bufs=1) as prod_pool:
                    tp_bf = [tp_pool.tile([P, d], BF16, name=f"tp{i}", tag=f"tp{i}")
                             for i in range(n_dblk)]
                    prodk = [prod_pool.tile([P, d], BF16, name=f"pk{i}", tag=f"pk{i}")
                             for i in range(n_dblk)]
                    prodq = [prod_pool.tile([P, d], BF16, name=f"pq{i}", tag=f"pq{i}")
                             for i in range(n_dblk)]

                    # T' = S Wk^T and R' = S Wq^T in one pass over wkq's 2048
                    # free columns.  T' kept in bf16 (lhsT of G); R' consumed
                    # directly from PSUM by the norm products.
                    with tc.tile_pool(name="tr_ps", bufs=8, space="PSUM") as tr_ps:
                        for ib in range(n_dblk):
                            pts = [tr_ps.tile([P, EF], F32, name="trps", tag="trps")
                                   for sp in range(4)]
                            for jc in range(n_dblk):
                                lh = s_bf[jc][:, ts(ib, P)]
                                for sp in range(4):
                                    nc.tensor.matmul(pts[sp], lhsT=lh,
                                                     rhs=wkq[jc][:, ts(sp, EF)],
                                                     start=(jc == 0), stop=(jc == n_dblk - 1))
                            for es in range(n_ef):
                                nc.vector.tensor_copy(out=tp_bf[ib][:, ts(es, EF)],
                                                      in_=pts[es])
                                nc.vector.tensor_mul(out=prodk[ib][:, ts(es, EF)],
                                                     in0=pts[es],
                                                     in1=wkq[ib][:, ts(es, EF)])
                                nc.vector.tensor_mul(out=prodq[ib][:, ts(es, EF)],
                                                     in0=pts[2 + es],
                                                     in1=wkq[ib][:, ts(2 + es, EF)])

                    # column norms of K and Q via ones-matmul partition sums
                    with tc.tile_pool(name="nrm_ps", bufs=4, space="PSUM") as nrm_ps, \
                         tc.tile_pool(name="misc_ps", bufs=2, space="PSUM") as misc_ps:
                        for prod, acc in ((prodk, nk2), (prodq, nq2)):
                            for es in range(n_ef):
                                pn = nrm_ps.tile([1, EF], F32, name="pn", tag="nrmps")
                                for ic in range(n_dblk):
                                    nc.tensor.matmul(pn, lhsT=ones_col,
                                                     rhs=prod[ic][:, ts(es, EF)],
                                                     start=(ic == 0), stop=(ic == n_dblk - 1))
                                nc.vector.tensor_copy(out=acc[:, ts(es, EF)], in_=pn)

                        nk = post_pool.tile([1, d], F32, name="nk", tag="nk")
                        nq = post_pool.tile([1, d], F32, name="nq", tag="nq")
                        nc.scalar.activation(out=nk, in_=nk2, func=AF.Sqrt)
                        nc.scalar.activation(out=nq, in_=nq2, func=AF.Sqrt)
                        nc.vector.tensor_scalar_add(out=nk, in0=nk, scalar1=1e-5)
                        nc.vector.tensor_scalar_add(out=nq, in0=nq, scalar1=1e-5)
                        nc.vector.reciprocal(out=nk, in_=nk)
                        nc.vector.reciprocal(out=nq, in_=nq)

                        # rk to per-partition columns, rq broadcast across
                        # partitions — both transposed on the PE.
                        for db in range(n_dblk):
                            pt = misc_ps.tile([P, 1], F32, name="pt", tag="miscps")
                            nc.tensor.matmul(pt, lhsT=nk[0:1, ts(db, P)], rhs=one11,
                                             start=True, stop=True)
                            nc.vector.tensor_copy(out=rk_col[:, db:db + 1], in_=pt)
                        for es in range(n_ef):
                            pb = misc_ps.tile([P, EF], F32, name="pb", tag="miscps_b")
                            nc.tensor.matmul(pb, lhsT=ones_row, rhs=nq[0:1, ts(es, EF)],
                                             start=True, stop=True)
                            nc.vector.tensor_copy(out=rq_bc[:, ts(es, EF)], in_=pb)

                    # G = T'^T Wq^T ; A = Prelu(rk * G, 0.25) in bf16
                    with tc.tile_pool(name="g_ps", bufs=6, space="PSUM") as g_ps:
                        for db in range(n_dblk):
                            for es in range(n_ef):
                                pg = g_ps.tile([P, EF], F32, name="pg", tag="gps")
                                for ic in range(n_dblk):
                                    nc.tensor.matmul(pg, lhsT=tp_bf[ic][:, ts(db, P)],
                                                     rhs=wkq[ic][:, ts(2 + es, EF)],
                                                     start=(ic == 0), stop=(ic == n_dblk - 1))
                                nc.scalar.activation(out=a_bf[db][:, ts(es, EF)], in_=pg,
                                                     func=AF.Prelu,
                                                     scale=rk_col[:, db:db + 1],
                                                     alpha=0.25)

            # ---------------- Pass 2: V, output ----------------
            with tc.tile_pool(name="xt2", bufs=3) as xt_pool, \
                 tc.tile_pool(name="vt", bufs=2) as vt_pool, \
                 tc.tile_pool(name="vrsb", bufs=10) as vr_pool, \
                 tc.tile_pool(name="gtmp", bufs=6) as gtmp_pool, \
                 tc.tile_pool(name="gate", bufs=10) as gate_pool, \
                 tc.tile_pool(name="osb", bufs=4) as osb_pool, \
                 tc.tile_pool(name="vrvi_ps", bufs=4, space="PSUM") as vrvi_ps, \
                 tc.tile_pool(name="out_ps", bufs=3, space="PSUM") as out_ps:

                xt_tiles = {}

                def dma_chunk(c):
                    tl = []
                    for dc in range(n_dblk):
                        t = xt_pool.tile([P, NC], BF16, name=f"xt{dc}", tag=f"xt{dc}")
                        nc.sync.dma_start(out=t, in_=xt_d[ts(dc, P), ts(c, NC)])
                        tl.append(t)
                    xt_tiles[c] = tl

                vt_tiles = {}

                def vrvi_chunk(c):
                    xtt = xt_tiles.pop(c)
                    vrs, sps = [], []
                    for eb in range(n_dblk):
                        pvr = vrvi_ps.tile([P, NC], F32, name="pvr", tag="vrvips")
                        pvi = vrvi_ps.tile([P, NC], F32, name="pvi", tag="vrvips")
                        for dc in range(n_dblk):
                            st = dict(start=(dc == 0), stop=(dc == n_dblk - 1))
                            nc.tensor.matmul(pvr, lhsT=wvr[dc][:, ts(eb, P)], rhs=xtt[dc], **st)
                            nc.tensor.matmul(pvi, lhsT=wvi[dc][:, ts(eb, P)], rhs=xtt[dc], **st)
                        # u = exp(vi)  (frees pvi; exp table stays loaded
                        # across all 8 tiles of the chunk)
                        sp = gtmp_pool.tile([P, NC], F32, name="sp", tag="sp")
                        nc.scalar.activation(out=sp, in_=pvi, func=AF.Exp)
                        sps.append(sp)
                        vr = vr_pool.tile([P, NC], BF16, name="vr", tag="vr")
                        nc.vector.tensor_copy(out=vr, in_=pvr)
                        vrs.append(vr)
                    # softplus = ln(1 + u), then gate = tanh(softplus) —
                    # batched so the ln/tanh tables each load once per chunk.
                    lns = []
                    for eb in range(n_dblk):
                        ln_t = gtmp_pool.tile([P, NC], F32, name="ln", tag="ln")
                        nc.scalar.activation(out=ln_t, in_=sps[eb], func=AF.Ln, bias=1.0)
                        lns.append(ln_t)
                    vts = []
                    for eb in range(n_dblk):
                        g = gate_pool.tile([P, NC], BF16, name="g", tag="g")
                        nc.scalar.activation(out=g, in_=lns[eb], func=AF.Tanh)
                        vt_t = vt_pool.tile([P, NC], BF16, name=f"vt{eb}", tag=f"vt{eb}")
                        nc.vector.tensor_mul(out=vt_t, in0=vrs[eb], in1=g)
                        vts.append(vt_t)
                    vt_tiles[c] = vts

                def out_chunk(c):
                    vts = vt_tiles.pop(c)
                    for s in range(NC // P):
                        for es in range(n_ef):
                            po = out_ps.tile([P, EF], F32, name="po", tag="ops")
                            for dc in range(n_dblk):
                                nc.tensor.matmul(po, lhsT=vts[dc][:, ts(s, P)],
                                                 rhs=a_bf[dc][:, ts(es, EF)],
                                                 start=(dc == 0), stop=(dc == n_dblk - 1))
                            ot = osb_pool.tile([P, EF], F32, name="ot", tag="osb")
                            nc.vector.tensor_mul(out=ot, in0=po, in1=rq_bc[:, ts(es, EF)])
                            nc.sync.dma_start(
                                out=out_d[c * NC + s * P:c * NC + (s + 1) * P, ts(es, EF)],
                                in_=ot)

                dma_chunk(0)
                dma_chunk(1)
                for c in range(n_chunks):
                    if c + 2 < n_chunks:
                        dma_chunk(c + 2)
                    vrvi_chunk(c)
                    if c > 0:
                        out_chunk(c - 1)
                out_chunk(n_chunks - 1)

    nc.compile()
    return nc


_PROGRAM_CACHE = {}


def _get_program(n, d):
    key = (n, d)
    if key not in _PROGRAM_CACHE:
        _PROGRAM_CACHE[key] = build_program(n, d)
    return _PROGRAM_CACHE[key]


def _numpy_reference(x, Wvr, bvr, Wvi, bvi, Wk, bk, Wq, bq):
    """Slow fp32 fallback (never expected to run: biases are zeros)."""
    out = np.empty_like(x)
    for b in range(x.shape[0]):
        xb = x[b].astype(np.float64)
        vr = xb @ Wvr.T.astype(np.float64) + bvr
        vi = xb @ Wvi.T.astype(np.float64) + bvi
        v = vr * np.tanh(np.logaddexp(0.0, vi))
        k = xb @ Wk.T.astype(np.float64) + bk
        q = xb @ Wq.T.astype(np.float64) + bq
        kn = k / (np.linalg.norm(k, axis=0, keepdims=True) + 1e-5)
        qn = q / (np.linalg.norm(q, axis=0, keepdims=True) + 1e-5)
        g = kn.T @ qn
        a = 0.625 * g + 0.375 * np.abs(g)
        out[b] = (v @ a).astype(np.float32)
    return out


def kernel(_run_kwargs=None, **inputs):
    run_kwargs = _run_kwargs or {}
    x = np.asarray(inputs["x"], dtype=np.float32)
    Wvr = np.asarray(inputs["Wvr"], dtype=np.float32)
    Wvi = np.asarray(inputs["Wvi"], dtype=np.float32)
    Wk = np.asarray(inputs["Wk"], dtype=np.float32)
    Wq = np.asarray(inputs["Wq"], dtype=np.float32)
    bvr, bvi = np.asarray(inputs["bvr"]), np.asarray(inputs["bvi"])
    bk, bq = np.asarray(inputs["bk"]), np.asarray(inputs["bq"])

    if any(np.any(b != 0) for b in (bvr, bvi, bk, bq)):
        return _numpy_reference(x, Wvr, bvr, Wvi, bvi, Wk, bk, Wq, bq)

    b, n, d = x.shape
    assert b == B and n == N_FULL and d == D_FULL, (b, n, d)

    bf16 = ml_dtypes.bfloat16
    wkq_h = np.ascontiguousarray(
        np.concatenate([Wk.T, Wq.T], axis=1)).astype(bf16)
    wvr_t = np.ascontiguousarray(Wvr.T).astype(bf16)
    wvi_t = np.ascontiguousarray(Wvi.T).astype(bf16)

    in_maps = []
    for i in range(N_CORES):
        in_maps.append({
            "xn": x[i].astype(bf16),
            "xt": np.ascontiguousarray(x[i].T).astype(bf16),
            "wkq": wkq_h, "wvr": wvr_t, "wvi": wvi_t,
        })

    nc = _get_program(n, d)
    from concourse.bass_utils import run_bass_kernel_spmd
    res = run_bass_kernel_spmd(nc, in_maps, core_ids=list(range(N_CORES)), **run_kwargs)
    out = np.stack([res.results[i]["out"] for i in range(N_CORES)], axis=0)
    if run_kwargs:
        kernel.last_results = res
    return out


# revision 9
# speedup vs baseline: 1.6068x; 1.0489x over previous
"""Trainium2 Bass kernel for nn_GatedFeedForward (gated feed-forward with
feature attention).

Reference computation per batch b (B=8, N=4096, D=1024):
    VR = x @ Wvr.T ; VI = x @ Wvi.T
    V  = VR * tanh(softplus(VI))
    K  = x @ Wk.T  ; Q  = x @ Wq.T
    Kn = K / (||K||_col + 1e-5) ; Qn = Q / (||Q||_col + 1e-5)   (norm over N)
    A  = smu(Kn.T @ Qn)          # smu == leaky-relu(slope 0.25) at mu=1e6
    out = V @ A
Sharding: pure data-parallel over batch — 8 batches on 8 NeuronCores.

Math restructure vs the naive 6-GEMM formulation (~45% less PE work):
  * Gram trick: K^T Q = Wk (x^T x) Wq^T.  With S = x^T x computed once,
    K and Q are never materialized:
        T' = S Wk^T ; R' = S Wq^T ; G = T'^T Wq^T   (S symmetric)
  * S is symmetric, so only the upper block-triangle is computed on the PE
    (12 of 16 [128,512] tiles); the mirrored blocks come from PE transposes.
  * Column norms from diagonals: ||K_d||^2 = diag(Wk S Wk^T)[d]
        = sum_i T'[i,d] * Wk^T[i,d]  -> elementwise product + ones-matmul.
  * smu == leaky_relu(0.25) exactly (erf saturates); leaky is positively
    homogeneous so rk folds into A rows (Prelu scale) and rq into the final
    output columns.
  * gate = tanh(softplus(vi)) evaluated as a fitted 3-term tanh mixture
        0.5 + sum_i ci * tanh(ai*vi + bi)      (max abs error 4.5e-5)
    so the scalar engine only ever uses ONE activation table in pass 2 (no
    act-table thrash from the tile scheduler) and no reciprocal is needed.

All matmuls bf16 with fp32 PSUM accumulation.
"""

import numpy as np
import ml_dtypes

import concourse.bass as bass
import concourse.tile as tile
import concourse.masks as masks
from concourse import bacc, mybir
from concourse.bass import ts

F32 = mybir.dt.float32
BF16 = mybir.dt.bfloat16
AF = mybir.ActivationFunctionType
ALU = mybir.AluOpType

B = 8
N_FULL = 4096
D_FULL = 1024
N_CORES = 8

P = 128   # SBUF partitions
NC = 512  # pass-2 sequence chunk
EF = 512  # free-dim span per matmul / psum bank

# tanh-mixture fit of tanh(softplus(x)) on [-12, 12]; limits are exact by
# construction (c1+c2+c3 = 1/2), max abs err 4.5e-5.
GC1, GA1, GB1 = 0.09744992, 0.45812075, 0.46371324
GC2, GA2, GB2 = 0.79134246, 0.60889040, -0.12094467
GC3, GA3, GB3 = 0.5 - GC1 - GC2, 0.58198337, -0.41616684


def build_program(n=N_FULL, d=D_FULL):
    """Build the single-core SPMD Bass program for one [n, d] batch."""
    assert n % NC == 0 and d % P == 0
    n_chunks = n // NC          # 8
    n_dblk = d // P             # 8 feature blocks
    n_ef = d // EF              # 2 spans of the feature dim
    n_xt = n // P               # 32 row-tiles of x

    nc = bacc.Bacc("TRN2", target_bir_lowering=False, debug=False,
                   num_devices=N_CORES)
    xn_d = nc.dram_tensor("xn", [n, d], BF16, kind="ExternalInput")
    xt_d = nc.dram_tensor("xt", [d, n], BF16, kind="ExternalInput")
    wkq_d = nc.dram_tensor("wkq", [d, 2 * d], BF16, kind="ExternalInput")
    wvr_d = nc.dram_tensor("wvr", [d, d], BF16, kind="ExternalInput")
    wvi_d = nc.dram_tensor("wvi", [d, d], BF16, kind="ExternalInput")
    out_d = nc.dram_tensor("out", [n, d], F32, kind="ExternalOutput")

    with tile.TileContext(nc) as tc:
        with tc.tile_pool(name="const", bufs=1) as const_pool, \
             tc.tile_pool(name="wv", bufs=1) as wv_pool, \
             tc.tile_pool(name="sbf", bufs=1) as sbf_pool, \
             tc.tile_pool(name="abf", bufs=1) as abf_pool, \
             tc.tile_pool(name="post", bufs=1) as post_pool:

            ones_col = const_pool.tile([P, 1], BF16, name="ones_col", tag="ones_col")
            nc.vector.memset(ones_col, 1.0)
            one11 = const_pool.tile([1, 1], F32, name="one11", tag="one11")
            nc.vector.memset(one11, 1.0)
            ones_row = const_pool.tile([1, P], F32, name="ones_row", tag="ones_row")
            nc.vector.memset(ones_row, 1.0)
            ident = const_pool.tile([P, P], BF16, name="ident", tag="ident")
            masks.make_identity(nc, ident[:])
            gate_bias = []
            for i, bv in enumerate((GB1, GB2, GB3)):
                t = const_pool.tile([P, 1], F32, name=f"gb{i}", tag=f"gb{i}")
                nc.vector.memset(t, bv)
                gate_bias.append(t)

            s_bf = [sbf_pool.tile([P, d], BF16, name=f"s{j}", tag=f"s{j}")
                    for j in range(n_dblk)]
            a_bf = [abf_pool.tile([P, d], BF16, name=f"a{j}", tag=f"a{j}")
                    for j in range(n_dblk)]

            # ---------------- Pass 1: S = x^T x (upper block-triangle) -----
            with tc.tile_pool(name="wkq", bufs=1) as wkq_pool:
                with tc.tile_pool(name="xtile", bufs=1) as xn_pool:
                    # x DMAs first — S matmuls need them immediately; weights
                    # follow in the queues (not used until P1.5/P2).  Each
                    # tile is split across two queues to halve arrival time.
                    xn = []
                    for ch in range(n_xt):
                        t = xn_pool.tile([P, d], BF16, name=f"x{ch}", tag=f"x{ch}")
                        nc.sync.dma_start(out=t[:, 0:EF], in_=xn_d[ts(ch, P), 0:EF])
                        nc.sync.dma_start(out=t[:, EF:d], in_=xn_d[ts(ch, P), EF:d])
                        xn.append(t)
                    wkq = []
                    for dc in range(n_dblk):
                        t = wkq_pool.tile([P, 2 * d], BF16, name=f"wkq{dc}", tag=f"wkq{dc}")
                        nc.sync.dma_start(out=t, in_=wkq_d[ts(dc, P), :])
                        wkq.append(t)
                    wvr, wvi = [], []
                    for wname, wdram, wl in (("wvr", wvr_d, wvr), ("wvi", wvi_d, wvi)):
                        for dc in range(n_dblk):
                            t = wv_pool.tile([P, d], BF16, name=f"{wname}{dc}", tag=f"{wname}{dc}")
                            nc.sync.dma_start(out=t, in_=wdram[ts(dc, P), :])
                            wl.append(t)

                    with tc.tile_pool(name="s_ps", bufs=8, space="PSUM") as s_ps:
                        # wave A: left half columns, row-tiles 0-3 (blocks on
                        # or above the diagonal); wave B: right half, all rows.
                        for es, ibs in ((0, range(4)), (1, range(8))):
                            ps = {ib: s_ps.tile([P, EF], F32, name="sps", tag="sps")
                                  for ib in ibs}
                            for ch in range(n_xt):
                                for ib in ibs:
                                    nc.tensor.matmul(ps[ib], lhsT=xn[ch][:, ts(ib, P)],
                                                     rhs=xn[ch][:, ts(es, EF)],
                                                     start=(ch == 0), stop=(ch == n_xt - 1))
                            for ib in ibs:
                                nc.vector.tensor_copy(out=s_bf[ib][:, ts(es, EF)],
                                                      in_=ps[ib])
                        # mirror the lower-left quadrant: S[4+r block, c block]
                        # = S[c block, 4+r block]^T for r,c in 0..3
                        for r in range(4):
                            for c in range(4):
                                pt = s_ps.tile([P, P], BF16, name="sps_t", tag="sps")
                                nc.tensor.transpose(pt, s_bf[c][:, EF + r * P:EF + (r + 1) * P],
                                                    ident)
                                nc.vector.tensor_copy(out=s_bf[4 + r][:, ts(c, P)], in_=pt)

                # ---------------- Pass 1.5: T'/R', norms, G -> A ----------
                rk_col = post_pool.tile([P, n_dblk], F32, name="rk_col", tag="rk_col")
                rq_bc = post_pool.tile([P, d], F32, name="rq_bc", tag="rq_bc")

                with tc.tile_pool(name="tp", bufs=1) as tp_pool, \
                     tc.tile_pool(name="prod", bufs=1) as prod_pool:
                    tp_bf = [tp_pool.tile([P, d], BF16, name=f"tp{i}", tag=f"tp{i}")
                             for i in range(n_dblk)]
                    prodk = [prod_pool.tile([P, d], BF16, name=f"pk{i}", tag=f"pk{i}")
                             for i in range(n_dblk)]
                    prodq = [prod_pool.tile([P, d], BF16, name=f"pq{i}", tag=f"pq{i}")
                             for i in range(n_dblk)]

                    # T' = S Wk^T and R' = S Wq^T in one pass over wkq's 2048
                    # free columns.  T' kept in bf16 (lhsT of G); R' consumed
                    # directly from PSUM by the norm products.  Row-blocks
                    # 4..7 first: they depend only on wave-B originals, so the
                    # mirror transposes can complete in their shadow.
                    with tc.tile_pool(name="tr_ps", bufs=8, space="PSUM") as tr_ps:
                        for ib in list(range(4, 8)) + list(range(4)):
                            pts = [tr_ps.tile([P, EF], F32, name="trps", tag="trps")
                                   for sp in range(4)]
                            for jc in range(n_dblk):
                                lh = s_bf[jc][:, ts(ib, P)]
                                for sp in range(4):
                                    nc.tensor.matmul(pts[sp], lhsT=lh,
                                                     rhs=wkq[jc][:, ts(sp, EF)],
                                                     start=(jc == 0), stop=(jc == n_dblk - 1))
                            for es in range(n_ef):
                                nc.vector.tensor_copy(out=tp_bf[ib][:, ts(es, EF)],
                                                      in_=pts[es])
                                nc.vector.tensor_mul(out=prodk[ib][:, ts(es, EF)],
                                                     in0=pts[es],
                                                     in1=wkq[ib][:, ts(es, EF)])
                                nc.vector.tensor_mul(out=prodq[ib][:, ts(es, EF)],
                                                     in0=pts[2 + es],
                                                     in1=wkq[ib][:, ts(2 + es, EF)])

                    # column norms of K and Q via ones-matmul partition sums;
                    # reciprocals run on full-partition tiles (a [1,1024]
                    # DVE reciprocal would serialize on one lane at ~6.4us).
                    nk2 = post_pool.tile([1, d], F32, name="nk2", tag="nk2")
                    nq2 = post_pool.tile([1, d], F32, name="nq2", tag="nq2")
                    with tc.tile_pool(name="nrm_ps", bufs=4, space="PSUM") as nrm_ps, \
                         tc.tile_pool(name="misc_ps", bufs=2, space="PSUM") as misc_ps:
                        for prod, acc in ((prodk, nk2), (prodq, nq2)):
                            for es in range(n_ef):
                                pn = nrm_ps.tile([1, EF], F32, name="pn", tag="nrmps")
                                for ic in range(n_dblk):
                                    nc.tensor.matmul(pn, lhsT=ones_col,
                                                     rhs=prod[ic][:, ts(es, EF)],
                                                     start=(ic == 0), stop=(ic == n_dblk - 1))
                                nc.vector.tensor_copy(out=acc[:, ts(es, EF)], in_=pn)

                        # rk: transpose nk2 to per-partition columns, then
                        # sqrt/recip on [128, 8] (partition-parallel).
                        nk2c = post_pool.tile([P, n_dblk], F32, name="nk2c", tag="nk2c")
                        for db in range(n_dblk):
                            pt = misc_ps.tile([P, 1], F32, name="pt", tag="miscps")
                            nc.tensor.matmul(pt, lhsT=nk2[0:1, ts(db, P)], rhs=one11,
                                             start=True, stop=True)
                            nc.vector.tensor_copy(out=nk2c[:, db:db + 1], in_=pt)
                        nc.scalar.activation(out=rk_col, in_=nk2c, func=AF.Sqrt)
                        nc.vector.tensor_scalar_add(out=rk_col, in0=rk_col, scalar1=1e-5)
                        nc.vector.reciprocal(out=rk_col, in_=rk_col)

                        # rq: broadcast nq2 across partitions first, then
                        # sqrt/recip on [128, 1024].
                        for es in range(n_ef):
                            pb = misc_ps.tile([P, EF], F32, name="pb", tag="miscps_b")
                            nc.tensor.matmul(pb, lhsT=ones_row, rhs=nq2[0:1, ts(es, EF)],
                                             start=True, stop=True)
                            nc.scalar.activation(out=rq_bc[:, ts(es, EF)], in_=pb,
                                                 func=AF.Sqrt)
                        nc.vector.tensor_scalar_add(out=rq_bc, in0=rq_bc, scalar1=1e-5)
                        nc.vector.reciprocal(out=rq_bc, in_=rq_bc)

                    # G = T'^T Wq^T ; A = Prelu(rk * G, 0.25) in bf16
                    with tc.tile_pool(name="g_ps", bufs=6, space="PSUM") as g_ps:
                        for db in range(n_dblk):
                            for es in range(n_ef):
                                pg = g_ps.tile([P, EF], F32, name="pg", tag="gps")
                                for ic in range(n_dblk):
                                    nc.tensor.matmul(pg, lhsT=tp_bf[ic][:, ts(db, P)],
                                                     rhs=wkq[ic][:, ts(2 + es, EF)],
                                                     start=(ic == 0), stop=(ic == n_dblk - 1))
                                nc.scalar.activation(out=a_bf[db][:, ts(es, EF)], in_=pg,
                                                     func=AF.Prelu,
                                                     scale=rk_col[:, db:db + 1],
                                                     alpha=0.25)

            # ---------------- Pass 2: V, output ----------------
            with tc.tile_pool(name="xt2", bufs=3) as xt_pool, \
                 tc.tile_pool(name="vt", bufs=2) as vt_pool, \
                 tc.tile_pool(name="gtmp", bufs=10) as gtmp_pool, \
                 tc.tile_pool(name="osb", bufs=4) as osb_pool, \
                 tc.tile_pool(name="vrvi_ps", bufs=4, space="PSUM") as vrvi_ps, \
                 tc.tile_pool(name="out_ps", bufs=3, space="PSUM") as out_ps:

                xt_tiles = {}

                def dma_chunk(c):
                    tl = []
                    for dc in range(n_dblk):
                        t = xt_pool.tile([P, NC], BF16, name=f"xt{dc}", tag=f"xt{dc}")
                        nc.sync.dma_start(out=t, in_=xt_d[ts(dc, P), ts(c, NC)])
                        tl.append(t)
                    xt_tiles[c] = tl

                vt_tiles = {}

                def vrvi_chunk(c):
                    xtt = xt_tiles.pop(c)
                    vts = []
                    for eb in range(n_dblk):
                        pvr = vrvi_ps.tile([P, NC], F32, name="pvr", tag="vrvips")
                        pvi = vrvi_ps.tile([P, NC], F32, name="pvi", tag="vrvips")
                        for dc in range(n_dblk):
                            st = dict(start=(dc == 0), stop=(dc == n_dblk - 1))
                            nc.tensor.matmul(pvr, lhsT=wvr[dc][:, ts(eb, P)], rhs=xtt[dc], **st)
                            nc.tensor.matmul(pvi, lhsT=wvi[dc][:, ts(eb, P)], rhs=xtt[dc], **st)
                        # gate = 0.5 + sum ci*tanh(ai*vi+bi); Tanh is the only
                        # scalar func in pass 2 -> single act table, no loads.
                        t1 = gtmp_pool.tile([P, NC], F32, name="t1", tag="gt")
                        t2 = gtmp_pool.tile([P, NC], F32, name="t2", tag="gt")
                        t3 = gtmp_pool.tile([P, NC], F32, name="t3", tag="gt")
                        nc.scalar.activation(out=t1, in_=pvi, func=AF.Tanh, scale=GA1,
                                             bias=gate_bias[0])
                        nc.scalar.activation(out=t2, in_=pvi, func=AF.Tanh, scale=GA2,
                                             bias=gate_bias[1])
                        nc.scalar.activation(out=t3, in_=pvi, func=AF.Tanh, scale=GA3,
                                             bias=gate_bias[2])
                        m1 = gtmp_pool.tile([P, NC], F32, name="m1", tag="gt")
                        nc.vector.scalar_tensor_tensor(out=m1, in0=t2, scalar=GC2 / GC1,
                                                       in1=t1, op0=ALU.mult, op1=ALU.add)
                        m2 = gtmp_pool.tile([P, NC], F32, name="m2", tag="gt")
                        nc.vector.scalar_tensor_tensor(out=m2, in0=t3, scalar=GC3 / GC1,
                                                       in1=m1, op0=ALU.mult, op1=ALU.add)
                        g = gtmp_pool.tile([P, NC], F32, name="g", tag="gt")
                        nc.vector.tensor_scalar(out=g, in0=m2, scalar1=GC1, scalar2=0.5,
                                                op0=ALU.mult, op1=ALU.add)
                        vt_t = vt_pool.tile([P, NC], BF16, name=f"vt{eb}", tag=f"vt{eb}")
                        nc.vector.tensor_mul(out=vt_t, in0=g, in1=pvr)
                        vts.append(vt_t)
                    vt_tiles[c] = vts

                def out_chunk(c):
                    vts = vt_tiles.pop(c)
                    for s in range(NC // P):
                        for es in range(n_ef):
                            po = out_ps.tile([P, EF], F32, name="po", tag="ops")
                            for dc in range(n_dblk):
                                nc.tensor.matmul(po, lhsT=vts[dc][:, ts(s, P)],
                                                 rhs=a_bf[dc][:, ts(es, EF)],
                                                 start=(dc == 0), stop=(dc == n_dblk - 1))
                            ot = osb_pool.tile([P, EF], F32, name="ot", tag="osb")
                            nc.vector.tensor_mul(out=ot, in0=po, in1=rq_bc[:, ts(es, EF)])
                            r0 = c * NC + s * P
                            nc.sync.dma_start(out=out_d[r0:r0 + P, ts(2 * es, EF // 2)],
                                              in_=ot[:, 0:EF // 2])
                            nc.sync.dma_start(out=out_d[r0:r0 + P, ts(2 * es + 1, EF // 2)],
                                              in_=ot[:, EF // 2:EF])

                dma_chunk(0)
                dma_chunk(1)
                for c in range(n_chunks):
                    if c + 2 < n_chunks:
                        dma_chunk(c + 2)
                    vrvi_chunk(c)
                    if c > 0:
                        out_chunk(c - 1)
                out_chunk(n_chunks - 1)

    nc.compile()
    return nc


_PROGRAM_CACHE = {}


def _get_program(n, d):
    key = (n, d)
    if key not in _PROGRAM_CACHE:
        _PROGRAM_CACHE[key] = build_program(n, d)
    return _PROGRAM_CACHE[key]


def _numpy_reference(x, Wvr, bvr, Wvi, bvi, Wk, bk, Wq, bq):
    """Slow fp32 fallback (never expected to run: biases are zeros)."""
    out = np.empty_like(x)
    for b in range(x.shape[0]):
        xb = x[b].astype(np.float64)
        vr = xb @ Wvr.T.astype(np.float64) + bvr
        vi = xb @ Wvi.T.astype(np.float64) + bvi
        v = vr * np.tanh(np.logaddexp(0.0, vi))
        k = xb @ Wk.T.astype(np.float64) + bk
        q = xb @ Wq.T.astype(np.float64) + bq
        kn = k / (np.linalg.norm(k, axis=0, keepdims=True) + 1e-5)
        qn = q / (np.linalg.norm(q, axis=0, keepdims=True) + 1e-5)
        g = kn.T @ qn
        a = 0.625 * g + 0.375 * np.abs(g)
        out[b] = (v @ a).astype(np.float32)
    return out


def kernel(_run_kwargs=None, **inputs):
    run_kwargs = _run_kwargs or {}
    x = np.asarray(inputs["x"], dtype=np.float32)
    Wvr = np.asarray(inputs["Wvr"], dtype=np.float32)
    Wvi = np.asarray(inputs["Wvi"], dtype=np.float32)
    Wk = np.asarray(inputs["Wk"], dtype=np.float32)
    Wq = np.asarray(inputs["Wq"], dtype=np.float32)
    bvr, bvi = np.asarray(inputs["bvr"]), np.asarray(inputs["bvi"])
    bk, bq = np.asarray(inputs["bk"]), np.asarray(inputs["bq"])

    if any(np.any(b != 0) for b in (bvr, bvi, bk, bq)):
        return _numpy_reference(x, Wvr, bvr, Wvi, bvi, Wk, bk, Wq, bq)

    b, n, d = x.shape
    assert b == B and n == N_FULL and d == D_FULL, (b, n, d)

    bf16 = ml_dtypes.bfloat16
    wkq_h = np.ascontiguousarray(
        np.concatenate([Wk.T, Wq.T], axis=1)).astype(bf16)
    wvr_t = np.ascontiguousarray(Wvr.T).astype(bf16)
    wvi_t = np.ascontiguousarray(Wvi.T).astype(bf16)

    in_maps = []
    for i in range(N_CORES):
        in_maps.append({
            "xn": x[i].astype(bf16),
            "xt": np.ascontiguousarray(x[i].T).astype(bf16),
            "wkq": wkq_h, "wvr": wvr_t, "wvi": wvi_t,
        })

    nc = _get_program(n, d)
    from concourse.bass_utils import run_bass_kernel_spmd
    res = run_bass_kernel_spmd(nc, in_maps, core_ids=list(range(N_CORES)), **run_kwargs)
    out = np.stack([res.results[i]["out"] for i in range(N_CORES)], axis=0)
    if run_kwargs:
        kernel.last_results = res
    return out
